# revision 5
# baseline (speedup 1.0000x reference)
"""DYConv2d (dynamic conv with rank-1 4D attention) on 8 Trainium2 cores.

bf16 conv datapath: image + synthesized weights in bf16 (matmuls stream
at ~189ns/448px with weight loads fully hidden vs 210ns for fp32r, and
image DMA halves). Attention MLP stays f32/f32r; PSUM accumulation and
output remain f32.

Startup path (sample 0) is latency-tuned: the image owns the single
sync DMA queue until it lands (startup is HBM-bound; queue-splitting
only divides bandwidth), channel sums run in parallel on DVE (ct0,
reduce_sum, fed the early chunks) and the ACT engine (ct1,
activation-Copy accum_out, fed the late chunks); weight synthesis
reads the rank-1 outer product directly from PSUM (ct0 on DVE) / an
ACT-copied SBUF mirror (ct1 on GpSimd, which cannot touch PSUM) with
single-tap-first splits so the first conv matmul issues ~4us after the
image lands.

Data-parallel over batch: each core takes 4 samples, synthesizes its
per-sample conv weights on device, and runs the per-sample 3x3 conv as
implicit GEMM (9 shifted matmuls x 2 C-halves accumulated in PSUM).

Self-contained: hardcodes all shapes; host side only reshapes/pads
inputs, shards across cores, and concatenates the per-core outputs.
"""

import numpy as np

B, C, O, KS, H, W, R = 32, 256, 256, 3, 56, 56, 16
KK = KS * KS  # 9
NCORES = 8
BL = B // NCORES  # 4 samples per core
WP = W + 2  # 58 (host-padded width)
HP = H + 2  # 58 (vertical pad rows live only in SBUF)
NPIX = HP * WP  # 3364
RG = 7  # row groups per image
RGH = 8  # output rows per group
NT = RGH * W  # 448 matmul free dim (<=512 fp32 PSUM bank)

TRACE = False
LAST_EXEC_NS = None
LAST_RESULTS = None

_CACHED = None


def _build_program():
    """Build + compile the per-core Bass program (cached)."""
    global _CACHED
    if _CACHED is not None:
        return _CACHED

    from contextlib import ExitStack

    from concourse import bacc
    import concourse.mybir as mybir
    import concourse.tile as tile

    f32 = mybir.dt.float32
    f32r = mybir.dt.float32r
    bf16 = mybir.dt.bfloat16
    AF = mybir.ActivationFunctionType
    AX = mybir.AxisListType

    nc = bacc.Bacc("TRN2", target_bir_lowering=False, debug=False)

    x_d = nc.dram_tensor("x", [BL, C, HP, WP], bf16, kind="ExternalInput").ap()
    bwT_d = nc.dram_tensor("bwT", [2, 128, KK * O], bf16, kind="ExternalInput").ap()
    fcsh_d = nc.dram_tensor("fcsh", [2, 128, R], bf16, kind="ExternalInput").ap()
    bsh_d = nc.dram_tensor("bsh", [R, 1], f32, kind="ExternalInput").ap()
    fcik_d = nc.dram_tensor("fcik", [R + 1, C + KK + 1], bf16, kind="ExternalInput").ap()
    hones_d = nc.dram_tensor("hones", [R + 1, 2], bf16, kind="ExternalInput").ap()
    fcoupT_d = nc.dram_tensor("fcoupT", [R + 1, O], bf16, kind="ExternalInput").ap()
    out_d = nc.dram_tensor("out", [BL, O, H, W], f32, kind="ExternalOutput").ap()

    with tile.TileContext(nc) as tc, ExitStack() as ctx:
        persist = ctx.enter_context(tc.tile_pool(name="persist", bufs=1))
        conv_psum = ctx.enter_context(
            tc.tile_pool(name="conv_psum", bufs=4, space="PSUM")
        )
        attn_psum = ctx.enter_context(
            tc.tile_pool(name="attn_psum", bufs=3, space="PSUM")
        )
        out_pool = ctx.enter_context(tc.tile_pool(name="out_pool", bufs=4))

        # warm-up buffer: memset (no DMA dependency) so the junk matmuls
        # start right after the preamble, ramping the PE pstate while the
        # first image streams; junk activations preload the ACT tables
        warm = persist.tile([128, 128 + NT], bf16, tag="warm", name="warm")
        nc.vector.memset(warm[:], 0.125)
        warm_act = persist.tile([1, 2], f32, tag="warm_act", name="warm_act")
        nc.scalar.activation(warm_act[:, 0:1], warm[:1, 0:1], AF.Relu)
        nc.scalar.activation(warm_act[:, 1:2], warm[:1, 0:1], AF.Sigmoid)
        warm_ps = conv_psum.tile([128, NT], f32, tag="warm_ps", name="warm_ps", bufs=1)

        def emit_warm_mms(n):
            for _ in range(n):
                nc.tensor.matmul(
                    warm_ps[:], warm[:, 0:128], warm[:, 128 : 128 + NT],
                    start=True, stop=True,
                )

        # ---- double-buffered per-sample image tiles ----
        ximg, ximg_v = [], []
        for s in range(2):
            ximg.append([persist.tile([128, NPIX], bf16, tag=f"ximg{s}{ct}", name=f"ximg{s}{ct}") for ct in range(2)])
            ximg_v.append([t[:].rearrange("p (r c) -> p r c", r=HP) for t in ximg[s]])

        HH = NPIX // 2  # 1682 = 29 rows; DMA halves so each channel-sum
        # engine can start on a half while the next streams

        # chunk row splits, decreasing: the channel-sum chain's tail is
        # last-chunk-land + that chunk's sum time, so the last chunks are
        # tiny; ct0 chunks lead each pair since DVE is the slower reducer
        CK_ROWS = (29, 19, 7, 3)
        CK_OFF = (0, 29, 48, 55)

        def emit_img_dma(s, b):
            # all chunks on the single sync queue (multi-queue attempts only
            # scrambled completion order), interleaved ct0/ct1 so both sum
            # engines start early
            insts = []

            def chunk(ct, i):
                insts.append(nc.sync.dma_start(
                    ximg[s][ct][:, CK_OFF[i] * WP : (CK_OFF[i] + CK_ROWS[i]) * WP],
                    x_d[b, ct * 128 : (ct + 1) * 128,
                        CK_OFF[i] : CK_OFF[i] + CK_ROWS[i], :]
                    .rearrange("p r c -> p (r c)"),
                ))

            for i in range(4):
                chunk(0, i)
                chunk(1, i)
            return insts

        img0_dmas = emit_img_dma(0, 0)
        emit_warm_mms(24)

        # ---- static weights (loaded once), ordered by first-use time;
        # they queue behind the image chunks on both queues
        h_ext = []
        for s in range(2):
            h_ext.append(persist.tile([R + 1, 2], bf16, tag=f"hext{s}", name=f"hext{s}"))
            nc.sync.dma_start(h_ext[s][:], hones_d[:])
        fcsh_sb = []
        for ct in range(2):
            t = persist.tile([128, R], bf16, tag=f"fcsh{ct}", name=f"fcsh{ct}")
            nc.sync.dma_start(t[:], fcsh_d[ct])
            fcsh_sb.append(t)
        bsh_sb = persist.tile([R, 1], f32, tag="bsh", name="bsh_sb")
        nc.sync.dma_start(bsh_sb[:], bsh_d[:])
        fcik_sb = persist.tile([R + 1, C + KK + 1], bf16, tag="fcik", name="fcik_sb")
        nc.sync.dma_start(fcik_sb[:], fcik_d[:])
        fcoupT_sb = persist.tile([R + 1, O], bf16, tag="fcoupT", name="fcoupT_sb")
        nc.sync.dma_start(fcoupT_sb[:], fcoupT_d[:])
        # bwT ct0 rides the idle gpsimd queue, sequenced to start right
        # after the image's ct0 chunks so it lands during the attention MLP
        # instead of queueing behind the statics; bwT ct1 drains the sync
        # queue concurrently — both land before weight synthesis needs them
        from concourse.tile import add_dep_helper

        bwT_sb = [
            persist.tile([128, KK * O], bf16, tag=f"bwT{ct}", name=f"bwT{ct}")
            for ct in range(2)
        ]
        di = nc.gpsimd.dma_start(bwT_sb[0][:], bwT_d[0])
        add_dep_helper(di.ins, img0_dmas[5].ins,
                       reason="bwT0 transfers once sample-0 image nearly landed")
        nc.sync.dma_start(bwT_sb[1][:], bwT_d[1])

        # scratch sink for the ACT-engine channel sums (accum_out carries
        # the result; the elementwise copy output is discarded)
        actsum = persist.tile([128, HH], bf16, tag="actsum", name="actsum")

        # ---- double-buffered per-sample state (slot = b % 2) ----
        w_sb, s_col, s_part, cs_p = [], [], [], [None, None]
        ainp_row, aoup_sb, colsc1_sb = [], [], []
        for s in range(2):
            w_sb.append([persist.tile([128, KK * O], bf16, tag=f"wsb{s}{ct}", name=f"wsb{s}{ct}") for ct in range(2)])
            s_col.append([persist.tile([128, 2], bf16, tag=f"scol{s}{ct}", name=f"scol{s}{ct}") for ct in range(2)])
            s_part.append([persist.tile([128, 6], f32, tag=f"spart{s}{ct}", name=f"spart{s}{ct}") for ct in range(2)])
            ainp_row.append(persist.tile([1, C + KK + 1], bf16, tag=f"ainp{s}", name=f"ainp{s}"))
            aoup_sb.append(persist.tile([128, 2], f32, tag=f"aoup{s}", name=f"aoup{s}"))
            colsc1_sb.append(persist.tile([128, KK], bf16, tag=f"colsc1{s}", name=f"colsc1{s}"))

        def emit_stage_b(s, b):
            # per-channel sums (pad zeros don't affect them) -> h = relu(...)
            # ct0 halves on DVE (reduce_sum), ct1 halves on the ACT engine
            # (Copy with accum_out): the two halves sum in parallel
            for i in range(4):
                nc.vector.reduce_sum(
                    s_part[s][0][:, i : i + 1],
                    ximg[s][0][:, CK_OFF[i] * WP : (CK_OFF[i] + CK_ROWS[i]) * WP],
                    axis=AX.X,
                )
            for i in range(4):
                nc.scalar.activation(
                    actsum[:, 0 : CK_ROWS[i] * WP],
                    ximg[s][1][:, CK_OFF[i] * WP : (CK_OFF[i] + CK_ROWS[i]) * WP],
                    AF.Copy,
                    accum_out=s_part[s][1][:, i : i + 1],
                )
            for ct in range(2):
                nc.vector.reduce_sum(
                    s_part[s][ct][:, 4:5], s_part[s][ct][:, 0:4], axis=AX.X
                )
                nc.vector.tensor_copy(
                    s_col[s][ct][:], s_part[s][ct][:, 4:5].broadcast_to((128, 2))
                )
            hp = attn_psum.tile([R, 2], f32, tag="apsum", name="hp")
            nc.tensor.matmul(hp[:], fcsh_sb[0][:], s_col[s][0][:], start=True, stop=False)
            nc.tensor.matmul(hp[:], fcsh_sb[1][:], s_col[s][1][:], start=False, stop=True)
            nc.scalar.activation(h_ext[s][0:R, :], hp[:], AF.Relu, bias=bsh_sb[:])

        def emit_stage_c(s, b):
            ainp_p = attn_psum.tile([2, C + KK + 1], f32, tag="apsum", name="ainp_p")
            nc.tensor.matmul(ainp_p[:], h_ext[s][:], fcik_sb[:], start=True, stop=True)
            nc.scalar.activation(ainp_row[s][:], ainp_p[0:1, :], AF.Sigmoid)

        def emit_stage_f(s, b):
            # a_oup is only consumed by the PSUM evacuation scale, so it sits
            # off the pre-conv critical path
            for ot in range(2):
                ao_p = attn_psum.tile([128, 2], f32, tag="apsum", name="ao_p")
                nc.tensor.matmul(
                    ao_p[:],
                    fcoupT_sb[:, ot * 128 : (ot + 1) * 128],
                    h_ext[s][:],
                    start=True,
                    stop=True,
                )
                nc.scalar.activation(aoup_sb[s][:, ot : ot + 1], ao_p[:, 0:1], AF.Sigmoid)

        def emit_stage_d(s, b):
            # colsc[c, k] = a_inp[c] * a_k[k] (rank-1 outer product on PE);
            # ct0 stays in PSUM (DVE reads it directly); ct1 is copied to
            # SBUF by the ACT engine since GpSimd cannot access PSUM
            for ct in range(2):
                cs = attn_psum.tile([128, KK + 1], f32, tag="apsum", name="cs_p")
                nc.tensor.matmul(
                    cs[:],
                    ainp_row[s][:, ct * 128 : (ct + 1) * 128],
                    ainp_row[s][:, C : C + KK + 1],
                    start=True,
                    stop=True,
                )
                cs_p[ct] = cs
            nc.scalar.activation(colsc1_sb[s][:], cs_p[1][:, 0:KK], AF.Copy)

        def emit_stage_e(s, b):
            # w[c, k*O+o] = base_wT[c, k*O+o] * colsc[c, k]; ct0 on DVE,
            # ct1 on GpSimd; single taps first so the first conv matmuls
            # (which consume taps in order) start with minimum latency
            for ct, eng, csrc in (
                (0, nc.vector, cs_p[0][:, 0:KK]),
                (1, nc.gpsimd, colsc1_sb[s][:]),
            ):
                wv = w_sb[s][ct][:].rearrange("p (k o) -> p k o", k=KK)
                bv = bwT_sb[ct][:].rearrange("p (k o) -> p k o", k=KK)
                for k0, kn in ((0, 1), (1, 1), (2, 1), (3, 3), (6, 3)):
                    eng.tensor_mul(
                        wv[:, k0 : k0 + kn, :],
                        bv[:, k0 : k0 + kn, :],
                        csrc[:, k0 : k0 + kn, None].broadcast_to(
                            (128, kn, O)
                        ),
                    )

        def emit_conv_group(s, b, ot, rg, split_evac=False):
            ps = conv_psum.tile([128, NT], f32, tag="cpsum", name="cps")
            first = True
            for ct in range(2):
                for kh in range(KS):
                    for kw in range(KS):
                        k = kh * KS + kw
                        nc.tensor.matmul(
                            ps[:],
                            w_sb[s][ct][
                                :, k * O + ot * 128 : k * O + ot * 128 + 128
                            ],
                            ximg_v[s][ct][
                                :, rg * RGH + kh : rg * RGH + kh + RGH, kw : kw + W
                            ],
                            start=first,
                            stop=(ct == 1 and k == KK - 1),
                        )
                        first = False
            osb = out_pool.tile([128, NT], f32, tag="osb", name="osb")
            # the very last group's evac+store is exposed in the tail, so
            # split it to overlap the ACT copy with the output DMA
            nh = 2 if split_evac else 1
            hw = NT // nh
            hr = RGH // nh
            for i in range(nh):
                nc.scalar.activation(
                    osb[:, i * hw : (i + 1) * hw],
                    ps[:, i * hw : (i + 1) * hw],
                    AF.Copy,
                    scale=aoup_sb[s][:, ot : ot + 1],
                )
                nc.sync.dma_start(
                    out_d[b, ot * 128 : (ot + 1) * 128,
                          rg * RGH + i * hr : rg * RGH + (i + 1) * hr, :],
                    osb[:, i * hw : (i + 1) * hw].rearrange(
                        "p (r c) -> p r c", r=hr
                    ),
                )

        # ---- pipeline (junk matmuls keep the PE warm through sample 0's
        # attention chain; later samples hide theirs under the previous conv)
        emit_stage_b(0, 0)
        emit_warm_mms(4)
        emit_stage_c(0, 0)
        emit_warm_mms(4)
        emit_stage_d(0, 0)
        emit_warm_mms(4)
        emit_stage_e(0, 0)
        emit_warm_mms(5)
        emit_stage_f(0, 0)
        for b in range(BL):
            s = b % 2
            sn = (b + 1) % 2
            gi = 0
            for ot in range(2):
                for rg in range(RG):
                    emit_conv_group(
                        s, b, ot, rg,
                        split_evac=(b == BL - 1 and ot == 1 and rg == RG - 1),
                    )
                    gi += 1
                    if b + 1 < BL:
                        if gi == 1:
                            emit_img_dma(sn, b + 1)
                        elif gi == 5:
                            emit_stage_b(sn, b + 1)
                        elif gi == 8:
                            emit_stage_c(sn, b + 1)
                        elif gi == 10:
                            emit_stage_d(sn, b + 1)
                        elif gi == 12:
                            emit_stage_e(sn, b + 1)
                        elif gi == 13:
                            emit_stage_f(sn, b + 1)

    nc.compile()
    _CACHED = nc
    return nc


def kernel(x, base_w, fc_share_w, fc_share_b, fc_inp_w, fc_inp_b,
           fc_oup_w, fc_oup_b, fc_k_w, fc_k_b):
    global LAST_EXEC_NS, LAST_RESULTS
    import ml_dtypes
    from concourse.bass_utils import run_bass_kernel_spmd

    nc = _build_program()

    bf = ml_dtypes.bfloat16
    x = np.asarray(x, np.float32)
    # host-side zero pad of H and W: every image DMA is one contiguous
    # chunk per partition and refreshes the pad border on each load
    xp = np.zeros((B, C, HP, WP), bf)
    xp[:, :, 1 : H + 1, 1 : W + 1] = x.astype(bf)

    bwT = np.ascontiguousarray(
        np.asarray(base_w, np.float32).transpose(1, 2, 3, 0).reshape(2, 128, KK * O)
    ).astype(bf)
    fcsh = np.ascontiguousarray(
        (np.asarray(fc_share_w, np.float32) / float(H * W)).T.reshape(2, 128, R)
    ).astype(bf)
    bsh = np.ascontiguousarray(np.asarray(fc_share_b, np.float32).reshape(R, 1))
    fcinT = np.concatenate([np.asarray(fc_inp_w, np.float32).T,
                            np.asarray(fc_inp_b, np.float32)[None, :]], axis=0)
    fckT = np.concatenate([np.asarray(fc_k_w, np.float32).T,
                           np.asarray(fc_k_b, np.float32)[None, :]], axis=0)
    fcik = np.ascontiguousarray(
        np.concatenate([fcinT, fckT, np.zeros((R + 1, 1), np.float32)], axis=1)
    ).astype(bf)
    hones = np.ones((R + 1, 2), bf)
    fcoupT = np.ascontiguousarray(
        np.concatenate([np.asarray(fc_oup_w, np.float32).T,
                        np.asarray(fc_oup_b, np.float32)[None, :]], axis=0)
    ).astype(bf)

    in_maps = []
    for i in range(NCORES):
        in_maps.append(
            {
                "x": np.ascontiguousarray(xp[i * BL : (i + 1) * BL]),
                "bwT": bwT,
                "fcsh": fcsh,
                "bsh": bsh,
                "fcik": fcik,
                "hones": hones,
                "fcoupT": fcoupT,
            }
        )

    res = run_bass_kernel_spmd(nc, in_maps, list(range(NCORES)), trace=TRACE)
    LAST_EXEC_NS = res.exec_time_ns
    LAST_RESULTS = res
    return np.concatenate([res.results[i]["out"] for i in range(NCORES)], axis=0)
